# revision 37
# baseline (speedup 1.0000x reference)
"""Trainium2 Bass kernel for nn_Attention_90752658965090.

Channel-attention restructuring: since attn is [c,c] with contraction over
n=4096, compute the Gram matrix Gx = x @ x.T once and fold GroupNorm +
qkv/proj weights into the [512,512] domain:

  logits = Wq D_s Gx D_s Wk^T + (Wq D_s xs) bk~^T + bq~ (Wk D_s xs)^T
           + n bq~ bk~^T          (bq~ = Wq t + bq, etc.)
  y      = x + (M D_s) x + r 1^T,  M = Wp D_z^-1 E Wv

where D_s/t are the per-channel GroupNorm scale/shift (stats come free from
diag(Gx) and row-sums xs), E = exp(scaled logits - max), D_z the softmax
denominators. x is read from HBM exactly once (resident in SBUF) and y
written once; total PE work is ~2.1x less than producing q/k/v explicitly.

Sharding: data-parallel over batch, 2 batch elements per core on 8 cores.
"""
import sys

sys.path.insert(0, "/opt/trn_rl_repo")

import numpy as np

import concourse.bass as bass
import concourse.mybir as mybir
import concourse.tile as tile
from concourse import bacc

B, C, HW = 16, 512, 4096
NCORES = 8
BPC = B // NCORES          # batches per core
P = 128
CT = C // P                # 4 channel tiles
NCH = HW // 512            # 8 n-chunks of 512
GROUPS = 8
EPS = 1e-5
INV_N = 1.0 / ((C // GROUPS) * HW)   # per-group element count
SCALE = float(C) ** -0.5

F32 = mybir.dt.float32
F32R = mybir.dt.float32r
AX = mybir.AxisListType
OP = mybir.AluOpType
AF = mybir.ActivationFunctionType


def frr(ap):
    return ap.bitcast(F32R)


def build_program(repeat=1):
    nc = bacc.Bacc("TRN2", target_bir_lowering=False, debug=False, num_devices=NCORES)

    x_d = nc.dram_tensor("x", [BPC, C, HW], F32R, kind="ExternalInput")
    y_d = nc.dram_tensor("y", [BPC, C, HW], F32, kind="ExternalOutput")
    wqkT_d = nc.dram_tensor("wqkT", [C, 2 * C], F32R, kind="ExternalInput")
    wvn_d = nc.dram_tensor("wvn", [C, C], F32R, kind="ExternalInput")
    wpT_d = nc.dram_tensor("wpT", [C, C], F32R, kind="ExternalInput")
    qkb_d = nc.dram_tensor("qkb", [1, 2 * C + 2], F32R, kind="ExternalInput")
    cols_d = nc.dram_tensor("cols", [P, 4 * CT], F32, kind="ExternalInput")
    indp_d = nc.dram_tensor("indp", [P, GROUPS * CT], F32, kind="ExternalInput")
    indT_d = nc.dram_tensor("indT", [GROUPS, C], F32, kind="ExternalInput")
    ident_d = nc.dram_tensor("ident", [P, P], F32R, kind="ExternalInput")

    from contextlib import ExitStack, nullcontext
    with tile.TileContext(nc) as tc, ExitStack() as ctx:
        wgt = ctx.enter_context(tc.tile_pool(name="wgt", bufs=1))
        xres = ctx.enter_context(tc.tile_pool(name="xres", bufs=48))
        xtp = ctx.enter_context(tc.tile_pool(name="xtp", bufs=3))
        # lifetime-disjoint [P,512] tiles share pools:
        #   pA: G1 (stats->U) / E (softmax->R) / Msb (M->SMT)
        #   pB: U (U->L) / R (R->M,r2)
        #   pC: WpZ (softmax->R) / SMT (SMT->final)
        pA = ctx.enter_context(tc.tile_pool(name="pA", bufs=CT))
        pB = ctx.enter_context(tc.tile_pool(name="pB", bufs=CT))
        pC = ctx.enter_context(tc.tile_pool(name="pC", bufs=2 * CT))
        gpool = upool = epool = rpool = mpool = smtp = wpzp = None  # via pA/pB/pC
        g0p = ctx.enter_context(tc.tile_pool(name="g0p", bufs=CT))
        ypool = ctx.enter_context(tc.tile_pool(name="ypool", bufs=6))
        rows = ctx.enter_context(tc.tile_pool(name="rows", bufs=4))
        sm = ctx.enter_context(tc.tile_pool(name="sm", bufs=8))
        dmp = ctx.enter_context(tc.tile_pool(name="dmp", bufs=2))
        psG = ctx.enter_context(tc.tile_pool(name="psG", bufs=CT,
                                             space=bass.MemorySpace.PSUM))
        psT = ctx.enter_context(tc.tile_pool(name="psT", bufs=2,
                                             space=bass.MemorySpace.PSUM))
        psM = ctx.enter_context(tc.tile_pool(name="psM", bufs=2,
                                             space=bass.MemorySpace.PSUM))

        # --- small constants first; big weight DMAs deferred so the x
        # stream (which gates the PE front) wins the DMA queue ---
        wq, wvn, wpT = [], [], []

        def load_wq():
            for t in range(CT):
                w1 = wgt.tile([P, 2 * C], F32R, tag=f"wq{t}", name=f"wq{t}")
                nc.sync.dma_start(w1[:], wqkT_d[t * P:(t + 1) * P, :])
                wq.append(w1)

        def load_wvp():
            for t in range(CT):
                w2 = wgt.tile([P, C], F32R, tag=f"wv{t}", name=f"wv{t}")
                nc.sync.dma_start(w2[:], wvn_d[t * P:(t + 1) * P, :])
                wvn.append(w2)
            for t in range(CT):
                w3 = wgt.tile([P, C], F32R, tag=f"wp{t}", name=f"wp{t}")
                nc.sync.dma_start(w3[:], wpT_d[t * P:(t + 1) * P, :])
                wpT.append(w3)

        identr = wgt.tile([P, P], F32R, tag="ident", name="identr")
        nc.sync.dma_start(identr[:], ident_d[:])
        epsg = wgt.tile([GROUPS, 1], F32, tag="epsg", name="epsg")
        nc.vector.memset(epsg[:], EPS)
        cols = wgt.tile([P, 4 * CT], F32, tag="cols", name="cols")
        indp = wgt.tile([P, GROUPS * CT], F32, tag="indp", name="indp")
        indT8 = wgt.tile([GROUPS, C], F32, tag="indT8", name="indT8")
        qkbr = wgt.tile([1, 2 * C + 2], F32R, tag="qkbr", name="qkbr")
        bvr = wgt.tile([P, 2 * CT], F32R, tag="bvr", name="bvr")

        def load_smalls():
            nc.sync.dma_start(cols[:], cols_d[:])
            nc.sync.dma_start(indp[:], indp_d[:])
            nc.sync.dma_start(indT8[:], indT_d[:])
            nc.sync.dma_start(qkbr[:], qkb_d[:])
            for t in range(CT):
                nc.vector.tensor_copy(bvr[:, 2 * t:2 * t + 1],
                                      cols[:, 4 * t + 2:4 * t + 3])
                nc.vector.tensor_copy(bvr[:, 2 * t + 1:2 * t + 2],
                                      cols[:, 4 * t + 2:4 * t + 3])

        one1 = qkbr[:, 2 * C:2 * C + 1]
        c4096 = qkbr[:, 2 * C + 1:2 * C + 2]
        nwc = [cols[:, 4 * t + 0:4 * t + 1] for t in range(CT)]
        nbc = [cols[:, 4 * t + 1:4 * t + 2] for t in range(CT)]
        vbc = [cols[:, 4 * t + 2:4 * t + 3] for t in range(CT)]
        pbc = [cols[:, 4 * t + 3:4 * t + 4] for t in range(CT)]
        indt = [indp[:, GROUPS * t:GROUPS * (t + 1)] for t in range(CT)]
        indTt = [indT8[:, t * P:(t + 1) * P] for t in range(CT)]

        # ---------- front: stream x, transpose, Gram, row-sums ----------
        class Front:
            def __init__(self, b):
                self.b = b
                self.Gps = [psG.tile([P, 512], F32, tag="g", name=f"G{b}_{t}")
                            for t in range(CT)]
                self.xsp = [sm.tile([P, NCH], F32, tag="xsp", name=f"xsp{b}_{t}")
                            for t in range(CT)]
                self.xc = {}
                self.pend = None
                self.ntg = 0

            def _gram(self, xt, last):
                # fp32r matmuls with <256-wide output run at 1/4 rate, so the
                # last block-row computes cols [256:512] (its lower block
                # (3,2) included) instead of a narrow 128-wide diagonal
                for ct, lo in ((0, 0), (1, P), (2, 2 * P), (3, 2 * P)):
                    nc.tensor.matmul(self.Gps[ct][:, lo:512],
                                     xt[:, ct * P:(ct + 1) * P],
                                     xt[:, lo:512], start=(self.ntg == 0),
                                     stop=last, skip_group_check=True)

            def chunk(self, ch, split=False):
                b = self.b
                xc = []
                for ct in range(CT):
                    t_ = xres.tile([P, 512], F32R, tag="xres",
                                   name=f"x{b}_{ch}_{ct}")
                    if split:
                        for h in range(2):
                            nc.sync.dma_start(
                                t_[:, h * 256:(h + 1) * 256],
                                x_d[b, ct * P:(ct + 1) * P,
                                    ch * 512 + h * 256:ch * 512 + (h + 1) * 256])
                    else:
                        nc.sync.dma_start(
                            t_[:],
                            x_d[b, ct * P:(ct + 1) * P, ch * 512:(ch + 1) * 512])
                    xc.append(t_)
                self.xc[ch] = xc
                for ct in range(CT):
                    nc.vector.reduce_sum(self.xsp[ct][:, ch:ch + 1],
                                         xc[ct].bitcast(F32)[:], axis=AX.X)
                for ns in range(4):
                    nt = ch * 4 + ns
                    tp = psT.tile([P, 512], F32R, tag="tp", name=f"tp{b}_{nt}")
                    for ct in range(CT):
                        nc.tensor.matmul(tp[:, ct * P:(ct + 1) * P],
                                         xc[ct][:, ns * P:(ns + 1) * P],
                                         identr[:], is_transpose=True,
                                         start=(ct == 0), stop=(ct == CT - 1),
                                         skip_group_check=True)
                    xt = xtp.tile([P, 512], F32R, tag="xt", name=f"xt{b}_{nt}")
                    nc.scalar.activation(xt[:], tp[:], AF.Copy)
                    if self.pend is not None:
                        self._gram(self.pend, last=False)
                        self.ntg += 1
                    self.pend = xt

            def finish(self):
                self._gram(self.pend, last=True)
                self.pend = None

        # ---------- stats: GroupNorm scale/shift + folded rows ----------
        def stats_p1(fr):
            b = fr.b
            # raw upper-triangular eviction releases the Gram PSUM banks early
            G0 = []
            for ct, lo in ((0, 0), (1, P), (2, 2 * P), (3, 2 * P)):
                g0 = g0p.tile([P, 512], F32R, tag="g0", name=f"g0_{b}_{ct}")
                nc.scalar.activation(g0[:, lo:512], fr.Gps[ct][:, lo:512],
                                     AF.Copy)
                G0.append(g0)
            st2 = []
            for ct in range(CT):
                dm = dmp.tile([P, P], F32, tag="dm", name=f"dm{b}_{ct}")
                nc.vector.tensor_tensor(dm[:],
                                        G0[ct][:, ct * P:(ct + 1) * P].bitcast(F32),
                                        identr.bitcast(F32)[:], op=OP.mult)
                s2 = sm.tile([P, 2], F32, tag="st2", name=f"st2_{b}_{ct}")
                nc.vector.reduce_sum(s2[:, 1:2], dm[:], axis=AX.X)
                nc.vector.reduce_sum(s2[:, 0:1], fr.xsp[ct][:], axis=AX.X)
                st2.append(s2)
            xs4 = sm.tile([P, CT], F32R, tag="xs4", name=f"xs4_{b}")
            for ct in range(CT):
                nc.vector.tensor_copy(xs4[:, ct:ct + 1], st2[ct][:, 0:1])
            return G0, (st2, xs4)

        def stats_p2(fr, G0, st2x, filler=None):
            st2, xs4 = st2x
            b = fr.b
            gp = psM.tile([GROUPS, 2], F32, tag="m", name=f"gp{b}")
            for ct in range(CT):
                nc.tensor.matmul(gp[:], indt[ct], st2[ct][:],
                                 start=(ct == 0), stop=(ct == CT - 1))
            gsb = sm.tile([GROUPS, 2], F32, tag="gsb", name=f"gsb{b}")
            nc.scalar.activation(gsb[:], gp[:], AF.Copy, scale=INV_N)
            m2 = sm.tile([GROUPS, 1], F32, tag="m2", name=f"m2_{b}")
            nc.vector.tensor_tensor(m2[:], gsb[:, 0:1], gsb[:, 0:1], op=OP.mult)
            var = sm.tile([GROUPS, 1], F32, tag="var", name=f"var{b}")
            nc.vector.tensor_tensor(var[:], gsb[:, 1:2], m2[:], op=OP.subtract)
            sq = sm.tile([GROUPS, 1], F32, tag="sq", name=f"sq{b}")
            nc.scalar.activation(sq[:], var[:], AF.Sqrt, bias=epsg[:])
            mrs = sm.tile([GROUPS, 2], F32, tag="mrs", name=f"mrs{b}")
            nc.vector.tensor_copy(mrs[:, 0:1], gsb[:, 0:1])
            nc.vector.reciprocal(mrs[:, 1:2], sq[:])
            if filler is not None:
                filler()   # PE work to cover the group-stat serial chain
            scl, bia, tdv, xsc = [], [], [], []
            for ct in range(CT):
                bp_ = psM.tile([P, 2], F32, tag="m", name=f"bps{b}_{ct}")
                nc.tensor.matmul(bp_[:], indTt[ct], mrs[:], start=True, stop=True)
                s_ = sm.tile([P, 1], F32, tag="scl", name=f"scl{b}_{ct}")
                nc.vector.tensor_tensor(s_[:], bp_[:, 1:2], nwc[ct], op=OP.mult)
                tmpb = sm.tile([P, 1], F32, tag="tmpb", name=f"tmpb{b}_{ct}")
                nc.vector.tensor_tensor(tmpb[:], bp_[:, 0:1], s_[:], op=OP.mult)
                b_ = sm.tile([P, 1], F32, tag="bia", name=f"bia{b}_{ct}")
                nc.vector.tensor_tensor(b_[:], nbc[ct], tmpb[:], op=OP.subtract)
                sinv = sm.tile([P, 1], F32, tag="sinv", name=f"sinv{b}_{ct}")
                nc.vector.reciprocal(sinv[:], s_[:])
                tdf = sm.tile([P, 1], F32, tag="tdf", name=f"tdf{b}_{ct}")
                nc.vector.tensor_tensor(tdf[:], b_[:], sinv[:], op=OP.mult)
                td_ = sm.tile([P, 2], F32R, tag="tdv", name=f"tdv{b}_{ct}")
                nc.vector.tensor_copy(td_[:, 0:1], tdf[:])
                nc.vector.tensor_copy(td_[:, 1:2], tdf[:])
                xsf = sm.tile([P, 1], F32, tag="xsf", name=f"xsf{b}_{ct}")
                nc.vector.tensor_tensor(xsf[:], s_[:], st2[ct][:, 0:1], op=OP.mult)
                t4 = sm.tile([P, 1], F32, tag="t4", name=f"t4_{b}_{ct}")
                nc.vector.tensor_scalar_mul(t4[:], b_[:], float(HW))
                uf = sm.tile([P, 1], F32, tag="uf", name=f"uf{b}_{ct}")
                nc.vector.tensor_tensor(uf[:], xsf[:], t4[:], op=OP.add)
                bx = sm.tile([P, 2], F32R, tag="bx2", name=f"bx2_{b}_{ct}")
                nc.vector.tensor_copy(bx[:, 0:1], b_[:])
                nc.vector.tensor_copy(bx[:, 1:2], uf[:])
                scl.append(s_); bia.append(bx); tdv.append(td_); xsc.append(bx)
            G1 = []
            for ct, lo in ((0, 0), (1, P), (2, 2 * P), (3, 2 * P)):
                g_ = pA.tile([P, 512], F32R, tag="pA", name=f"g1_{b}_{ct}")
                nc.scalar.activation(g_[:, lo:512], G0[ct][:, lo:512],
                                     AF.Copy, scale=scl[ct][:])
                G1.append(g_)
            for ct, ct2 in ((1, 0), (2, 0), (2, 1), (3, 0), (3, 1)):
                    tpm = psT.tile([P, P], F32R, tag="tp", name=f"mir{b}_{ct}_{ct2}")
                    nc.tensor.matmul(tpm[:, 0:P], G0[ct2][:, ct * P:(ct + 1) * P],
                                     identr[:], is_transpose=True, start=True,
                                     stop=True, skip_group_check=True)
                    nc.scalar.activation(G1[ct][:, ct2 * P:(ct2 + 1) * P],
                                         tpm[:, 0:P], AF.Copy, scale=scl[ct][:])
            # xs as a [1, 512] row (raw, for the U rank-1 term)
            xr_ = psT.tile([1, 512], F32, tag="tp", name=f"xr{b}")
            for ct in range(CT):
                nc.tensor.matmul(xr_[:, ct * P:(ct + 1) * P], xs4[:, ct:ct + 1],
                                 identr[:], start=(ct == 0),
                                 stop=(ct == CT - 1), skip_group_check=True)
            xs_row = rows.tile([1, 512], F32R, tag="rows", name=f"xsr{b}")
            nc.scalar.activation(xs_row[:], xr_[:], AF.Copy)
            # bq~ = Wq t + bq (one row); the k-half packs [t | s*xs+4096*t]
            # as a 2-col stationary, yielding bk~ AND w = Wk(s*xs)+4096*bk~
            # in one 512-col pass (bias rank-1 uses the adjacent [1,4096]
            # constants in qkbr directly)
            ps_ = psT.tile([1, 512], F32, tag="tp", name=f"bq{b}")
            for et in range(CT):
                nc.tensor.matmul(ps_[:], bia[et][:, 0:1],
                                 wq[et][:, 0:512], start=(et == 0), stop=False,
                                 skip_group_check=True)
            nc.tensor.matmul(ps_[:], one1, qkbr[:, 0:512],
                             start=False, stop=True, skip_group_check=True)
            bq_row = rows.tile([1, 512], F32R, tag="rows", name=f"brow{b}")
            nc.scalar.activation(bq_row[:], ps_[:], AF.Copy)
            kw_ = psT.tile([2, 512], F32, tag="tp", name=f"kw{b}")
            for et in range(CT):
                nc.tensor.matmul(kw_[:], bia[et][:], wq[et][:, 512:1024],
                                 start=(et == 0), stop=False, skip_group_check=True)
            nc.tensor.matmul(kw_[:], qkbr[:, 2 * C:2 * C + 2],
                             qkbr[:, 512:1024], start=False, stop=True,
                             skip_group_check=True)
            kwsb = rows.tile([2, 512], F32R, tag="rows", name=f"kwsb{b}")
            nc.scalar.activation(kwsb[:], kw_[:], AF.Copy)
            bk_row = kwsb[0:1, :]
            w_row = rows.tile([1, 512], F32R, tag="rows", name=f"wr{b}")
            nc.sync.dma_start(w_row[:], kwsb[1:2, :])
            return dict(scl=scl, tdv=tdv, G1=G1, xs_row=xs_row,
                        bq_row=bq_row, bk_row=bk_row, w_row=w_row)

        # ---------- U / L / softmax ----------
        def ul_softmax(b, st, filler=None):
            G1, scl = st["G1"], st["scl"]
            U = []
            for ft in range(CT):
                pU = psM.tile([P, 512], F32, tag="m", name=f"pU{b}_{ft}")
                for et in range(CT):
                    nc.tensor.matmul(pU[:], G1[et][:, ft * P:(ft + 1) * P],
                                     wq[et][:, 512:1024], start=(et == 0),
                                     stop=False, skip_group_check=True)
                nc.tensor.matmul(pU[:], st["xs_row"][:, ft * P:(ft + 1) * P],
                                 st["bk_row"][:], start=False, stop=True,
                                 skip_group_check=True)
                u_ = pB.tile([P, 512], F32R, tag="pB", name=f"u{b}_{ft}")
                nc.scalar.activation(u_[:], pU[:], AF.Copy, scale=scl[ft][:])
                U.append(u_)
            if filler is not None:
                filler()   # PE work to cover the U-eviction latency
            E, rz = [], []
            for qt in range(CT):
                pL = psM.tile([P, 512], F32, tag="m", name=f"pL{b}_{qt}")
                for ft in range(CT):
                    nc.tensor.matmul(pL[:], wq[ft][:, qt * P:(qt + 1) * P],
                                     U[ft][:], start=(ft == 0), stop=False,
                                     skip_group_check=True)
                nc.tensor.matmul(pL[:], st["bq_row"][:, qt * P:(qt + 1) * P],
                                 st["w_row"][:], start=False, stop=True,
                                 skip_group_check=True)
                nmx = sm.tile([P, 1], F32, tag="nmx", name=f"nmx{b}_{qt}")
                nc.vector.reduce_max(nmx[:], pL[:], axis=AX.X, negate=True)
                nms = sm.tile([P, 1], F32, tag="nms", name=f"nms{b}_{qt}")
                nc.vector.tensor_scalar_mul(nms[:], nmx[:], SCALE)
                e_ = pA.tile([P, 512], F32R, tag="pA", name=f"e{b}_{qt}")
                z_ = sm.tile([P, 1], F32, tag="z", name=f"z{b}_{qt}")
                nc.scalar.activation(e_[:], pL[:], AF.Exp, bias=nms[:],
                                     scale=SCALE, accum_out=z_[:])
                r_ = sm.tile([P, 1], F32, tag="rz", name=f"rz{b}_{qt}")
                nc.vector.reciprocal(r_[:], z_[:])
                E.append(e_); rz.append(r_)
            return E, rz

        # ---------- backend: R, M, SMT, r ----------
        def backend(b, st, E, rz, filler=None):
            scl, tdv = st["scl"], st["tdv"]
            WpZ = []
            for ct in range(CT):
                wz = pC.tile([P, 512], F32R, tag="pC", name=f"wpz{b}_{ct}")
                nc.scalar.activation(wz[:], wpT[ct][:], AF.Copy, scale=rz[ct][:])
                WpZ.append(wz)
            R = []
            for dt in range(CT):
                pR = psM.tile([P, 512], F32, tag="m", name=f"pR{b}_{dt}")
                for ct in range(CT):
                    nc.tensor.matmul(pR[:], E[ct][:, dt * P:(dt + 1) * P],
                                     WpZ[ct][:], start=(ct == 0),
                                     stop=(ct == CT - 1), skip_group_check=True)
                r_ = pB.tile([P, 512], F32R, tag="pB", name=f"r{b}_{dt}")
                nc.scalar.activation(r_[:], pR[:], AF.Copy)
                R.append(r_)
            if filler is not None:
                filler()   # PE work to cover the R-eviction latency
            # SMT[et] = s[e] * (Wp D^-1 E Wv)^T block: Wv^T R directly
            # (lhsT = wvn native slice is transposed by the PE).
            SMT = []
            for et in range(CT):
                pM = psM.tile([P, 512], F32, tag="m", name=f"pM{b}_{et}")
                for dt in range(CT):
                    nc.tensor.matmul(pM[:], wvn[dt][:, et * P:(et + 1) * P],
                                     R[dt][:], start=(dt == 0),
                                     stop=(dt == CT - 1), skip_group_check=True)
                s_ = pC.tile([P, 512], F32R, tag="pC", name=f"smt{b}_{et}")
                nc.scalar.activation(s_[:], pM[:], AF.Copy, scale=scl[et][:])
                SMT.append(s_)
            rcol = []
            for c2t in range(CT):
                pr = psM.tile([P, 2], F32, tag="m", name=f"pr{b}_{c2t}")
                for et in range(CT):
                    nc.tensor.matmul(pr[:], SMT[et][:, c2t * P:(c2t + 1) * P],
                                     tdv[et][:], start=(et == 0), stop=False,
                                     skip_group_check=True)
                for dt in range(CT):
                    nc.tensor.matmul(pr[:], R[dt][:, c2t * P:(c2t + 1) * P],
                                     bvr[:, 2 * dt:2 * dt + 2], start=False,
                                     stop=(dt == CT - 1), skip_group_check=True)
                rc = sm.tile([P, 1], F32, tag="rc", name=f"rc{b}_{c2t}")
                nc.scalar.activation(rc[:], pr[:, 0:1], AF.Identity, bias=pbc[c2t],
                                     scale=1.0)
                rcol.append(rc)
            return SMT, rcol

        # ---------- final streaming matmul + residual ----------
        def final_chunk(b, ch, SMT, rcol, xc):
            for ot in range(CT):
                pY = psM.tile([P, 512], F32, tag="m", name=f"pY{b}_{ch}_{ot}")
                for et in range(CT):
                    nc.tensor.matmul(pY[:], SMT[et][:, ot * P:(ot + 1) * P],
                                     xc[et][:], start=(et == 0),
                                     stop=(et == CT - 1), skip_group_check=True)
                yt = ypool.tile([P, 512], F32, tag="y", name=f"yt{b}_{ch}_{ot}")
                nc.vector.scalar_tensor_tensor(
                    out=yt[:], in0=pY[:], scalar=rcol[ot][:],
                    in1=xc[ot].bitcast(F32)[:], op0=OP.add, op1=OP.add)
                nc.sync.dma_start(
                    y_d[b, ot * P:(ot + 1) * P, ch * 512:(ch + 1) * 512], yt[:])

        rep_cm = tc.For_i(0, repeat, 1) if repeat > 1 else nullcontext()
        with rep_cm:
            fr0 = Front(0)
            fr0.chunk(0)
            load_smalls()
            for ch in range(1, NCH):
                fr0.chunk(ch)
            fr0.finish()
            # batch-1 front work is threaded through batch-0's serial
            # stats/softmax chains so the PE never starves
            G0_0, st2_0 = stats_p1(fr0)
            fr1 = Front(1)
            fr1.chunk(0)
            load_wq()          # q/k weights: needed first by stats_p2 rows
            fr1.chunk(1)
            load_wvp()         # v/proj weights needed only by the backend
            st0 = stats_p2(fr0, G0_0, st2_0)
            fr1.chunk(2)
            fr1.chunk(3)
            E0, rz0 = ul_softmax(0, st0)
            SMT0, rcol0 = backend(0, st0, E0, rz0)
            for ch in range(4):
                final_chunk(0, ch, SMT0, rcol0, fr0.xc[ch])
                fr1.chunk(ch + 4)
            fr1.finish()
            G0_1, st2_1 = stats_p1(fr1)
            final_chunk(0, 4, SMT0, rcol0, fr0.xc[4])
            st1 = stats_p2(fr1, G0_1, st2_1)
            final_chunk(0, 5, SMT0, rcol0, fr0.xc[5])
            E1, rz1 = ul_softmax(1, st1)
            final_chunk(0, 6, SMT0, rcol0, fr0.xc[6])
            SMT1, rcol1 = backend(1, st1, E1, rz1)
            final_chunk(0, 7, SMT0, rcol0, fr0.xc[7])
            for ch in range(NCH):
                final_chunk(1, ch, SMT1, rcol1, fr1.xc[ch])

    nc.compile()
    return nc


_NC = None


def _get_program():
    global _NC
    if _NC is None:
        _NC = build_program()
    return _NC


def make_in_maps(x, norm_w, norm_b, qkv_w, qkv_b, proj_w, proj_b):
    x = np.asarray(x, dtype=np.float32).reshape(B, C, HW)
    qkv_w = np.asarray(qkv_w, dtype=np.float32)
    proj_w = np.asarray(proj_w, dtype=np.float32)
    qkv_b = np.asarray(qkv_b, dtype=np.float32)
    nw = np.asarray(norm_w, np.float32).reshape(CT, P)
    nb = np.asarray(norm_b, np.float32).reshape(CT, P)
    vb = qkv_b[2 * C:].reshape(CT, P)
    pb = np.asarray(proj_b, np.float32).reshape(CT, P)
    cols = np.empty((P, 4 * CT), np.float32)
    for t in range(CT):
        cols[:, 4 * t + 0] = nw[t]
        cols[:, 4 * t + 1] = nb[t]
        cols[:, 4 * t + 2] = vb[t]
        cols[:, 4 * t + 3] = pb[t]
    ind = np.eye(GROUPS, dtype=np.float32)[np.arange(C) // (C // GROUPS)]  # [C, G]
    indp = np.empty((P, GROUPS * CT), np.float32)
    for t in range(CT):
        indp[:, GROUPS * t:GROUPS * (t + 1)] = ind[t * P:(t + 1) * P]
    common = {
        "wqkT": np.ascontiguousarray(qkv_w[:2 * C].T),
        "wvn": np.ascontiguousarray(qkv_w[2 * C:]),
        "wpT": np.ascontiguousarray(proj_w.T),
        "qkb": np.ascontiguousarray(
            np.concatenate([qkv_b[:2 * C],
                            np.array([1.0, float(HW)], np.float32)]
                           ).reshape(1, 2 * C + 2)),
        "cols": cols,
        "indp": indp,
        "indT": np.ascontiguousarray(ind.T),
        "ident": np.eye(P, dtype=np.float32),
    }
    return [
        {"x": np.ascontiguousarray(x[i * BPC:(i + 1) * BPC]), **common}
        for i in range(NCORES)
    ]


def _wait_device(max_wait=600):
    """The axon-tunneled device can be transiently unrecoverable right after
    another process's teardown; poll with a tiny op until it responds."""
    import time
    import jax
    import jax.numpy as jnp
    t0 = time.time()
    while True:
        try:
            v = float((jnp.ones((4, 4)) @ jnp.ones((4, 4))).sum())
            assert v == 64.0
            return
        except Exception:
            if time.time() - t0 > max_wait:
                raise
            time.sleep(30)


def run(inputs, trace=False):
    import time
    from concourse.bass_utils import run_bass_kernel_spmd
    nc = _get_program()
    in_maps = make_in_maps(**inputs)
    last_err = None
    for attempt in range(3):
        try:
            if attempt > 0:
                time.sleep(60)
            _wait_device()
            r = run_bass_kernel_spmd(nc, in_maps, list(range(NCORES)), trace=trace)
            break
        except Exception as e:
            last_err = e
    else:
        raise last_err
    y = np.concatenate([r.results[i]["y"] for i in range(NCORES)], axis=0)
    return y.reshape(B, C, 64, 64), r


def kernel(**inputs):
    y, _ = run(inputs, trace=False)
    return y
